# revision 43
# baseline (speedup 1.0000x reference)
"""Tensor-parallel GQA attention (Llama-3-8B shape, prefill, start_pos=0) on 8
Trainium2 NeuronCores.

Sharding: core i owns kv-head i and q-heads 4i..4i+3 — wq/wk/wv column-shards,
wo row-shard, x replicated.  Each core computes a partial [2048, 4096] output
(its heads pushed through its wo rows); the host sums the 8 partials
(all-reduce equivalent).

Layout (all matmul operands bf16, fp32 PSUM accumulate):
  - Stages interleaved per 512-wide seq chunk: A(sc) projections+RoPE, B(ic)
    attention, C(ic) wo-matmul.  Causal emission: A0 A1 B0 C0 A2 B1 A3
    B2{+C1} B3{+C2} C3 — C1/C2's wo-matmul groups are interleaved into the
    exp-bound stretches of B2/B3 (between each score matmul and its PV
    matmul) so the tensor engine rides through the ACT exp latency.
  - Stage A is out-tile-major (k, q0..q3, vT accumulate 32 contraction
    matmuls each in one PSUM bank).  Chunk 0 is cold (x still streaming
    from HBM): its first three tiles run kt-outer across 3 concurrent PSUM
    groups, consuming x quarters as they arrive.  vT transpose runs on the
    DMA XBAR (dma_start_transpose), not PE.
  - RoPE: rotate-half via partition-swap DMA; sin tables sign-folded, q
    tables pre-scaled by 1/sqrt(hd); k/q tables shared via stride-0
    broadcast APs; all products on DVE (2x bf16 path).
  - Scores transposed ST[j, i] = kT.T @ qT; exp on ACT; causal mask applied
    on PE as an additive -60 upper-tri bias matmul (m60 stationary x ident
    moving accumulated into the diagonal score block) — no vector-engine
    mask work and exp(masked) ~ e^-50 is harmless in the rowsums.
  - Softmax denominators: heads 0/3 (and all heads on short j-loops) use
    the classic all-ones stationary matmul accumulated alongside PV; on
    long j-loops heads 1/2 instead accumulate exp tiles elementwise (h1 on
    DVE; h2 split even/odd j-tiles across DVE and gpsimd) into f32 SBUF
    accumulators + a single 512-col ones-matmul partition reduce per head,
    software-pipelined into h3's j-loop.
  - Stage C accumulates 4 heads per (i-tile, ech) in PSUM (head order
    0,1,3,2 — h2's deferred normalize lands last), DVE-evicts 2-ech-packed
    bf16 tiles, DMAs on alternating queues; final tile goes per-ech so the
    drain only waits on a 128 KB transfer.  Host sums partials in f32.
  - DMA queues (both feed one HWDGE; a queue's sequencer shares its host
    engine, and a dep-waiting DMA at the head wedges that sequencer):
    sync/SP carries weights, rotate-half swaps, XBAR transposes, x for
    chunks 2/3 and half the outputs; scalar/ACT carries rope tables, x for
    chunks 0/1 (ACT idle then) and the other half of the outputs — so exp
    dispatch is never stuck behind a wedged descriptor ring.
  - PSUM budget (8 banks): psa 2 (A accumulators; B's deferred rowsum
    reduces) + st 3 (scores, shared with C's wo accumulators) + pv 2 + rs 1.
"""

import math
import os
from contextlib import ExitStack

import numpy as np

import concourse.bass as bass
import concourse.tile as tile
from concourse import bacc, mybir
from concourse.bass_utils import run_bass_kernel_spmd

# ---- problem shape (hardcoded per contract) ----
S = 2048           # seq len
D = 4096           # model dim
HD = 128           # head dim
N_CORES = 8
NQH = 4            # q heads per core
QCOLS = NQH * HD   # 512 wq columns per core
SC_N = 4           # seq chunks of 512
KT_N = D // 128    # 32 contraction tiles
JT_N = S // 128    # 16 key tiles
ECH_N = D // 512   # 8 output column chunks

F32 = mybir.dt.float32
BF16 = mybir.dt.bfloat16

MM_DT = BF16          # matmul operand dtype
USE_CHAINS = os.environ.get("K_CHAINS", "1") == "1"
PEP_BUFS = 12         # exp-tile ring (WAR slack for the DVE/Pool chains)

_BUILD_CACHE: dict = {}


def _make_pools(tc, ctx):
    def pool(name, bufs, space="SBUF"):
        return ctx.enter_context(tc.tile_pool(name=name, bufs=bufs, space=space))

    return {
        "wp": pool("w", 1),
        "xp": pool("x", 3),
        "rpp": pool("rope", 2),
        "kqp": pool("kq", 1),
        "rtp": pool("rt", 1),
        "pep": pool("pe", PEP_BUFS),
        "rcp": pool("rc", 2),
        "accp": pool("acc", 2),
        "obp": pool("ob", 3),
        "vfp": pool("vf", 2),
        "ps": pool("ps", 1, space="PSUM"),
    }


def _emit_body(nc, pools, dram, out, causal: bool):
    if True:
        wp = pools["wp"]
        xp = pools["xp"]
        rpp = pools["rpp"]
        kqp = pools["kqp"]
        rtp = pools["rtp"]
        pep = pools["pep"]
        rcp = pools["rcp"]
        accp = pools["accp"]
        obp = pools["obp"]
        vfp = pools["vfp"]
        ps = pools["ps"]

        # ---- resident weights & constants (DMA order = consumption order:
        # the cold-start A(0) phase consumes k/q0/q1 kt-outer immediately,
        # then q2, q3, v; rope 0/1 ride ahead of the big wo transfer) ----
        wk_sb = wp.tile([128, KT_N, HD], MM_DT, tag="wk", name="wk_sb")
        wq_sb = wp.tile([128, NQH, KT_N, HD], MM_DT, tag="wq", name="wq_sb")
        for i in range(4):
            ksl = slice(i * 8, (i + 1) * 8)
            nc.sync.dma_start(wk_sb[:, ksl, :], dram["wk"][:, ksl, :])
            nc.sync.dma_start(wq_sb[:, 0, ksl, :], dram["wq"][:, 0, ksl, :])
            nc.sync.dma_start(wq_sb[:, 1, ksl, :], dram["wq"][:, 1, ksl, :])
        nc.sync.dma_start(wq_sb[:, 2], dram["wq"][:, 2])
        nc.sync.dma_start(wq_sb[:, 3], dram["wq"][:, 3])
        wv_sb = wp.tile([128, KT_N, HD], MM_DT, tag="wv", name="wv_sb")
        nc.sync.dma_start(wv_sb[:], dram["wv"][:])
        ones_sb = wp.tile([128, 128], MM_DT, tag="ones", name="ones_sb")
        nc.sync.dma_start(ones_sb[:], dram["ones"][:])
        ident_sb = wp.tile([128, 128], MM_DT, tag="ident", name="ident_sb")
        nc.sync.dma_start(ident_sb[:], dram["ident"][:])
        # additive causal mask: m60[i, j] = -60 for j > i (else 0); applied on
        # PE as `st[:, blk] += m60.T @ ident` so no vector engine is involved
        m60_sb = wp.tile([128, 128], MM_DT, tag="m60", name="m60_sb")
        nc.sync.dma_start(m60_sb[:], dram["m60"][:])
        tri_sb = wp.tile([128, 128], MM_DT, tag="tri", name="tri_sb")
        nc.sync.dma_start(tri_sb[:], dram["tri"][:])

        # rope tables: rp[:, pl, t2, :] pl 0 = cos, 1 = sign-folded sin;
        # t2 0 = k table, 1 = q table (pre-scaled by 1/sqrt(hd)).
        # chunks 0/1 load ahead of wo; 2/3 load from their stage bodies
        rp_tiles = {}

        def load_rope(sc):
            rp = rpp.tile([HD, 2, 2, 512], MM_DT, tag="rp", name="rp")
            nc.scalar.dma_start(rp[:], dram["rope"][sc])
            rp_tiles[sc] = rp

        load_rope(0)
        load_rope(1)
        wo_sb = wp.tile([128, ECH_N, NQH, 512], MM_DT, tag="wo", name="wo_sb")
        for i in range(2):
            nc.sync.dma_start(
                wo_sb[:, i * 4:(i + 1) * 4, :, :], dram["wo"][:, i * 4:(i + 1) * 4, :, :]
            )

        # ---- per-chunk persistent activations ----
        # kqT[sc]: t=0 is kT, t=1..4 are qT (overwritten by attention output)
        kqT = [
            kqp.tile([128, 5, 512], MM_DT, tag=f"kq{sc}", name=f"kq{sc}")
            for sc in range(SC_N)
        ]
        v_sb = [
            kqp.tile([128, 4, HD], MM_DT, tag=f"v{sc}", name=f"v{sc}")
            for sc in range(SC_N)
        ]

        def stage_a(sc):
            # x for the chunk in two half-contraction tiles (bufs=3 gives a
            # half-chunk of prefetch ahead of compute)
            xlo = xp.tile([128, 16, 512], MM_DT, tag="xt", name="xlo")
            xhi = xp.tile([128, 16, 512], MM_DT, tag="xt", name="xhi")
            # chunks 0/1 stream x on the scalar queue (ACT is idle then);
            # later chunks ride the sync queue so their descriptor rings
            # never block exp dispatch on the ACT sequencer
            xq = nc.scalar if sc < 2 else nc.sync
            for i in range(4):
                xq.dma_start(
                    xlo[:, i * 4:(i + 1) * 4, :], dram["xn"][sc, i]
                )
            for i in range(4):
                xq.dma_start(
                    xhi[:, i * 4:(i + 1) * 4, :], dram["xn"][sc, 4 + i]
                )

            def xsl(kt):
                return (xlo if kt < 16 else xhi)[:, kt % 16, :]

            if sc not in rp_tiles:
                load_rope(sc)
            rp = rp_tiles[sc]

            def wsl(t, kt):
                return wk_sb[:, kt, :] if t == 0 else wq_sb[:, t - 1, kt, :]

            qc = rtp.tile([128, 5, 512], MM_DT, tag="qc", name="qc")
            # out-tile order: k, q0..q3 (rope group), then vT
            if sc == 0:
                # cold start: x is still streaming in, so pace the first
                # three tiles kt-outer (3 concurrent PSUM groups: psa x2 and
                # the otherwise-idle rs bank) to consume x as it arrives
                g = [
                    ps.tile([128, 512], F32, tag="psa", name="g0", bufs=2),
                    ps.tile([128, 512], F32, tag="psa", name="g1", bufs=2),
                    ps.tile([128, 512], F32, tag="rs", name="g2", bufs=1),
                ]
                for kt in range(KT_N):
                    for t in range(3):
                        nc.tensor.matmul(
                            g[t][:], wsl(t, kt), xsl(kt),
                            start=kt == 0, stop=kt == KT_N - 1,
                        )
                for t in range(3):
                    nc.vector.tensor_copy(qc[:, t, :], g[t][:])
                rest = range(3, 5)
            else:
                rest = range(5)
            for t in rest:
                pacc = ps.tile([128, 512], F32, tag="psa", name="psa", bufs=2)
                for kt in range(KT_N):
                    nc.tensor.matmul(
                        pacc[:],
                        wsl(t, kt),
                        xsl(kt),
                        start=kt == 0,
                        stop=kt == KT_N - 1,
                    )
                nc.vector.tensor_copy(qc[:, t, :], pacc[:])
            # rotate-half partition swap for all 5 tiles at once
            qs = rtp.tile([128, 5, 512], MM_DT, tag="qs", name="qs")
            nc.sync.dma_start(qs[0:64, :, :], qc[64:128, :, :])
            nc.sync.dma_start(qs[64:128, :, :], qc[0:64, :, :])
            # sin product (sign-folded tables) on DVE, cos product on gpsimd
            nc.vector.tensor_mul(qs[:, 0:1, :], qs[:, 0:1, :], rp[:, 1, 0:1, :])
            nc.vector.tensor_mul(
                qs[:, 1:5, :], qs[:, 1:5, :],
                rp[:, 1, 1:2, :].broadcast_to([HD, 4, 512]),
            )
            nc.vector.tensor_mul(qc[:, 0:1, :], qc[:, 0:1, :], rp[:, 0, 0:1, :])
            nc.vector.tensor_mul(
                qc[:, 1:5, :], qc[:, 1:5, :],
                rp[:, 0, 1:2, :].broadcast_to([HD, 4, 512]),
            )
            nc.vector.tensor_add(kqT[sc][:], qc[:], qs[:])

            # vT projection + transpose to v_sb[sc] ([j, d] layout) — the
            # transpose runs on the DMA XBAR, not the tensor engine
            pv = ps.tile([128, 512], F32, tag="psa", name="psav", bufs=2)
            for kt in range(KT_N):
                nc.tensor.matmul(
                    pv[:], wv_sb[:, kt, :], xsl(kt),
                    start=kt == 0, stop=kt == KT_N - 1,
                )
            vt_f = vfp.tile([128, 512], MM_DT, tag="vt", name="vt")
            nc.vector.tensor_copy(vt_f[:], pv[:])
            for vi in range(4):
                nc.sync.dma_start_transpose(
                    v_sb[sc][:, vi, :], vt_f[:, vi * 128:(vi + 1) * 128]
                )

        def stage_b(ic, cth=None):
            njt = 4 * (ic + 1) if causal else JT_N
            # chains only pay off on long j-loops; short chunks use the
            # classic per-j-tile PE rowsum for every head
            chains = USE_CHAINS and njt >= 12
            # interleave the previous chunk's wo-matmul groups into the
            # chain heads' j-loops: those are exp(ACT)-bound, so the pc
            # matmuls fill otherwise-idle PE cycles
            cth = list(cth) if cth else []
            cstride = 1
            cslot = [0]

            def pop_c():
                if cth:
                    cth.pop(0)()
            q12 = max(njt // 2, 2)
            q34 = max(3 * njt // 4, 3)
            # h1/h2 exp accumulators (f32) + staging
            acc1 = accp.tile([128, 512], F32, tag="acc", name="acc1")
            acc2e = accp.tile([128, 512], F32, tag="acc", name="acc2e")
            acc2o = accp.tile([128, 512], F32, tag="acco", name="acc2o", bufs=1)
            accb1 = accp.tile([128, 512], MM_DT, tag="accb", name="accb1")
            accb2 = accp.tile([128, 512], MM_DT, tag="accb", name="accb2")
            pvf1 = accp.tile([128, 512], F32, tag="pvf", name="pvf1")
            pvf2 = accp.tile([128, 512], F32, tag="pvf", name="pvf2")
            rc = [None] * NQH
            rsp = [None] * NQH
            # off of jt==1 (odd-subchain init) — nonzero only for ic==0
            co = 128 if (causal and ic == 0) else 0

            def mk_rc(h):
                t = rcp.tile([128, 512], F32, tag="rc", name=f"rc{h}")
                rc[h] = t
                return t

            def fin_reduce(h):
                # ones-matmul partition reduce of the accumulated exp sums
                accb = accb1 if h == 1 else accb2
                r = ps.tile([128, 512], F32, tag="psa", name=f"rsp{h}", bufs=2)
                rsp[h] = r
                nc.tensor.matmul(r[:], ones_sb[:], accb[:], start=True, stop=True)

            def fin_norm(h):
                nc.vector.reciprocal(mk_rc(h)[:], rsp[h][:])
                pvf = pvf1 if h == 1 else pvf2
                nc.vector.tensor_mul(kqT[ic][:, 1 + h, :], pvf[:], rc[h][:])

            def emit_point(h, jt):
                if not chains:
                    return
                if h == 2 and jt == 1:
                    # h1 chain fully emitted -> DVE-in-order guarantees done;
                    # early emission keeps it ahead of h2's chain backlog
                    nc.vector.tensor_copy(accb1[:], acc1[:])
                elif h == 3:
                    if jt == 0:
                        fin_reduce(1)
                    elif jt == 1:
                        fin_norm(1)
                    elif jt == q12:
                        # combine h2 odd subchain into even, convert
                        nc.vector.tensor_add(
                            acc2e[:, co:], acc2e[:, co:], acc2o[:, co:]
                        )
                        nc.vector.tensor_copy(accb2[:], acc2e[:])
                    elif jt == q34:
                        fin_reduce(2)
                        if q34 + 1 > njt - 1:
                            fin_norm(2)
                    elif jt == q34 + 1:
                        fin_norm(2)

            for h in range(NQH):
                classic = not chains or h in (0, 3)
                pv = ps.tile([128, 512], F32, tag="pv", name=f"pv{h}", bufs=2)
                rs = (
                    ps.tile([128, 512], F32, tag="rs", name=f"rs{h}", bufs=1)
                    if classic
                    else None
                )
                for jt in range(njt):
                    dt_ = jt - 4 * ic  # diagonal tile index (causal only)
                    diag = causal and dt_ >= 0
                    off = 128 * dt_ if (causal and dt_ > 0) else 0
                    blk = off + 128
                    kT = kqT[jt // 4][:, 0, (jt % 4) * 128:(jt % 4 + 1) * 128]
                    qT = kqT[ic][:, 1 + h, :]
                    st = ps.tile([128, 512], F32, tag="st", name="st", bufs=3)
                    if diag and USE_CHAINS:
                        # causal mask as additive -60 upper-tri bias on PE:
                        # score over the full range, mask accumulated onto the
                        # diagonal block only
                        nc.tensor.matmul(
                            st[:, off:], kT, qT[:, off:],
                            start=True, stop=False,
                        )
                        nc.tensor.matmul(
                            st[:, off:blk], m60_sb[:], ident_sb[:],
                            start=False, stop=True, skip_group_check=True,
                        )
                    else:
                        nc.tensor.matmul(
                            st[:, off:], kT, qT[:, off:], start=True, stop=True
                        )
                    pe = pep.tile([128, 512], MM_DT, tag="pe", name="pe")
                    nc.scalar.activation(
                        pe[:, off:], st[:, off:], mybir.ActivationFunctionType.Exp
                    )
                    if not classic:
                        # absorbed wo-matmul work sits between the score and
                        # the PV matmul, hiding the exp latency
                        cslot[0] += 1
                        if cslot[0] % cstride == 0:
                            pop_c()
                    if diag and not USE_CHAINS:
                        nc.vector.tensor_mul(
                            pe[:, off:blk], pe[:, off:blk], tri_sb[:]
                        )
                    first, last = jt == 0, jt == njt - 1
                    nc.tensor.matmul(
                        pv[:, off:], v_sb[jt // 4][:, jt % 4, :], pe[:, off:],
                        start=first, stop=last,
                    )
                    if classic:
                        nc.tensor.matmul(
                            rs[:, off:], ones_sb[:], pe[:, off:],
                            start=first, stop=last,
                        )
                    elif h == 1:
                        if first:
                            nc.vector.tensor_copy(acc1[:], pe[:])
                        else:
                            nc.vector.tensor_add(
                                acc1[:, off:], acc1[:, off:], pe[:, off:]
                            )
                    else:  # h == 2: even jts on DVE, odd jts on gpsimd
                        tgt, eng = (
                            (acc2e, nc.vector) if jt % 2 == 0 else (acc2o, nc.gpsimd)
                        )
                        if jt < 2:
                            eng.tensor_copy(tgt[:, off:], pe[:, off:])
                        else:
                            eng.tensor_add(
                                tgt[:, off:], tgt[:, off:], pe[:, off:]
                            )
                    emit_point(h, jt)
                # end of h's j-loop
                if classic:
                    nc.vector.reciprocal(mk_rc(h)[:], rs[:])
                    nc.vector.tensor_mul(kqT[ic][:, 1 + h, :], pv[:], rc[h][:])
                elif h == 1:
                    nc.vector.tensor_copy(pvf1[:], pv[:])
                else:
                    nc.vector.tensor_copy(pvf2[:], pv[:])
            # flush any wo-matmul groups not absorbed by the j-loops
            while cth:
                cth.pop(0)()

        def c_thunks(ic):
            # per-pc granularity: one thunk = one 4-head wo accumulation +
            # eviction, so each exp-bound B slot can absorb exactly one
            thunks = []
            for itl in range(4):
                it = 4 * ic + itl
                isl = slice(itl * 128, (itl + 1) * 128)
                # the final tile's output DMAs go out per-ech so the end-of-
                # kernel drain only waits on a 128 KB transfer
                fine = causal and ic == SC_N - 1 and itl == 3
                for eh in range(ECH_N // 2):
                    # pack 2 ech per ob tile; DVE eviction (gpsimd can't read
                    # PSUM), one DMA per pair on alternating queues
                    holder = {}

                    def th(e2, it=it, isl=isl, eh=eh, fine=fine, holder=holder):
                        if e2 == 0:
                            holder["ob"] = obp.tile(
                                [128, 2, 512], MM_DT, tag="ob", name="ob"
                            )
                        ob = holder["ob"]
                        ech = 2 * eh + e2
                        pc = ps.tile(
                            [128, 512], F32, tag="st", name="pc", bufs=3
                        )
                        # h2's softmax finalize lands latest -> read last
                        for hx, h in enumerate((0, 1, 3, 2)):
                            nc.tensor.matmul(
                                pc[:],
                                kqT[ic][:, 1 + h, isl],
                                wo_sb[:, ech, h, :],
                                start=hx == 0,
                                stop=hx == NQH - 1,
                            )
                        nc.vector.tensor_copy(ob[:, e2, :], pc[:])
                        if fine:
                            deng = nc.sync if (it + ech) % 2 else nc.scalar
                            deng.dma_start(out[it, eh][:, e2, :], ob[:, e2, :])
                        elif e2 == 1:
                            deng = nc.sync if (it + eh) % 2 else nc.scalar
                            deng.dma_start(out[it, eh], ob[:])

                    from functools import partial
                    thunks.append(partial(th, 0))
                    thunks.append(partial(th, 1))
            return thunks

        def stage_c(ic):
            for th in c_thunks(ic):
                th()

        if causal:
            # C1/C2 are absorbed into the exp-bound stretches of B2/B3
            stage_a(0)
            stage_a(1)
            stage_b(0)
            stage_c(0)
            stage_a(2)
            stage_b(1)
            stage_a(3)
            stage_b(2, c_thunks(1))
            stage_b(3, c_thunks(2))
            stage_c(3)
        else:
            # non-causal: every B chunk reads all chunks' K/V, so all of
            # stage A must be emitted first (a B read of a not-yet-written
            # kqT range would get no RAW dependency from Tile)
            for sc in range(SC_N):
                stage_a(sc)
            for ic in range(SC_N):
                stage_b(ic)
                stage_c(ic)


def build_nc(causal: bool = True, reps: int = 1):
    nc = bacc.Bacc(
        "TRN2", target_bir_lowering=False, debug=False, num_devices=N_CORES
    )
    dram = {}
    for name, shape, dt in [
        # host-prepermuted layouts: every DMA reads/writes contiguous
        # per-partition runs
        ("xn", [SC_N, 8, 128, 4, 512], MM_DT),
        ("wq", [128, NQH, KT_N, HD], MM_DT),
        ("wk", [128, KT_N, HD], MM_DT),
        ("wv", [128, KT_N, HD], MM_DT),
        ("wo", [128, ECH_N, NQH, 512], MM_DT),
        ("rope", [SC_N, HD, 2, 2, 512], MM_DT),
        ("ones", [128, 128], MM_DT),
        ("ident", [128, 128], MM_DT),
        ("m60", [128, 128], MM_DT),
        ("tri", [128, 128], MM_DT),
    ]:
        dram[name] = nc.dram_tensor(name, shape, dt, kind="ExternalInput").ap()
    out = nc.dram_tensor("out", [JT_N, ECH_N // 2, 128, 2, 512], MM_DT,
                         kind="ExternalOutput").ap()

    with tile.TileContext(nc) as tc:
        with ExitStack() as ctx:
            pools = _make_pools(tc, ctx)
            for _ in range(reps):
                _emit_body(nc, pools, dram, out, causal)

    nc.compile()
    return nc


def get_nc(causal: bool = True):
    if causal not in _BUILD_CACHE:
        _BUILD_CACHE[causal] = build_nc(causal)
    return _BUILD_CACHE[causal]


def _mm_np(a):
    return np.ascontiguousarray(a).astype(mybir.dt.np(MM_DT))


def prep_in_maps(x, sincos, wq, wk, wv, wo):
    """Host-side shard + layout prep. Returns list of per-core input dicts.

    All tensors are pre-permuted so that every device DMA moves contiguous
    per-partition runs (device DMA engines are far more efficient that way).
    """
    x = np.asarray(x, np.float32)
    assert x.shape == (1, S, D)
    # xn[sc, xg, p, k4, n] = x[sc*512 + n, (xg*4 + k4)*128 + p]
    xn = _mm_np(
        x[0].reshape(SC_N, 512, 8, 4, 128).transpose(0, 2, 4, 3, 1)
    )

    sincos = np.asarray(sincos, np.float32)
    sin = sincos[:S, :HD]
    cos = sincos[:S, HD:]
    sinT = np.ascontiguousarray(sin.T)
    cosT = np.ascontiguousarray(cos.T)
    sin_sgn = sinT.copy()
    sin_sgn[:64] = -sinT[:64]
    scale = np.float32(1.0 / math.sqrt(HD))
    # rope[sc, d, pl, t2, n]: pl 0 = cos, 1 = sin(signed); t2 0 = k, 1 = q
    base = np.stack([cosT, sin_sgn], axis=0)           # [2(pl), HD, S]
    rope = np.stack([base, base * scale], axis=1)      # [2(pl), 2(t2), HD, S]
    rope = rope.reshape(2, 2, HD, SC_N, 512).transpose(3, 2, 0, 1, 4)
    rope = _mm_np(rope)

    wq = np.asarray(wq, np.float32)
    wk = np.asarray(wk, np.float32)
    wv = np.asarray(wv, np.float32)
    wo = np.asarray(wo, np.float32)

    ones = _mm_np(np.ones((128, 128), np.float32))
    ident = _mm_np(np.eye(128, dtype=np.float32))
    m60 = _mm_np(np.triu(np.ones((128, 128), np.float32), 1) * np.float32(-60.0))
    tri = _mm_np(np.triu(np.ones((128, 128), np.float32)))

    in_maps = []
    for c in range(N_CORES):
        wq_c = wq[:, c * QCOLS:(c + 1) * QCOLS]          # [D, 512]
        wk_c = wk[:, c * HD:(c + 1) * HD]                # [D, 128]
        wv_c = wv[:, c * HD:(c + 1) * HD]
        wo_c = wo[c * QCOLS:(c + 1) * QCOLS, :]          # [512, D]
        in_maps.append(
            {
                "xn": xn,
                # wq[p, h, kt, d] = wq_c[kt*128 + p, h*HD + d]
                "wq": _mm_np(
                    wq_c.reshape(KT_N, 128, NQH, HD).transpose(1, 2, 0, 3)
                ),
                "wk": _mm_np(
                    wk_c.reshape(KT_N, 128, HD).transpose(1, 0, 2)
                ),
                "wv": _mm_np(
                    wv_c.reshape(KT_N, 128, HD).transpose(1, 0, 2)
                ),
                # wo[p, ech, h, n] = wo_c[h*128 + p, ech*512 + n]
                "wo": _mm_np(
                    wo_c.reshape(NQH, 128, ECH_N, 512).transpose(1, 2, 0, 3)
                ),
                "rope": rope,
                "ones": ones,
                "ident": ident,
                "m60": m60,
                "tri": tri,
            }
        )
    return in_maps


def unpermute_out(out_n):
    """out_n [it, eh, p, e2, n] -> out [S, D]."""
    return np.ascontiguousarray(
        out_n.transpose(0, 2, 1, 3, 4).reshape(S, D)
    )


def check_mask(full_causal_mask, start_pos) -> bool:
    """Returns True for causal (tril) mask, False for all-allowed."""
    sp = int(start_pos)
    assert sp == 0, f"kernel specialized for start_pos=0, got {sp}"
    m = np.asarray(full_causal_mask)
    assert m.shape == (1, 1, S, S)
    m = m[0, 0]
    tril = np.tril(np.ones((S, S), dtype=bool))
    if (m == tril).all():
        return True
    if m.all():
        return False
    raise AssertionError("unsupported mask pattern")


def kernel(
    x,
    start_pos,
    sincos,
    full_causal_mask,
    wq,
    wk,
    wv,
    wo,
    cache_k,
    cache_v,
):
    causal = check_mask(full_causal_mask, start_pos)
    # cache_k/cache_v are zero and fully overwritten in the attended region
    # (start_pos=0, seq_len == max_seq_len) — they do not affect the output.
    nc = get_nc(causal)
    in_maps = prep_in_maps(x, sincos, wq, wk, wv, wo)
    res = run_bass_kernel_spmd(nc, in_maps, list(range(N_CORES)))
    acc = res.results[0]["out"].astype(np.float32)
    for c in range(1, N_CORES):
        acc = acc + res.results[c]["out"].astype(np.float32)
    return unpermute_out(acc)[np.newaxis]


# revision 50
# speedup vs baseline: 1.0376x; 1.0376x over previous
"""Tensor-parallel GQA attention (Llama-3-8B shape, prefill, start_pos=0) on 8
Trainium2 NeuronCores.

Sharding: core i owns kv-head i and q-heads 4i..4i+3 — wq/wk/wv column-shards,
wo row-shard, x replicated.  Each core computes a partial [2048, 4096] output
(its heads pushed through its wo rows); the host sums the 8 partials
(all-reduce equivalent).

Layout (all matmul operands bf16, fp32 PSUM accumulate):
  - Stages interleaved per 512-wide seq chunk: A(sc) projections+RoPE, B(ic)
    attention, C(ic) wo-matmul.  Causal emission: A0 A1 B0 C0 A2 B1 A3
    B2{+C1} B3{+C2} C3 — C1/C2's wo-matmul groups are interleaved into the
    exp-bound stretches of B2/B3 (between each score matmul and its PV
    matmul) so the tensor engine rides through the ACT exp latency.
  - Stage A is out-tile-major (k, q0..q3, vT accumulate 32 contraction
    matmuls each in one PSUM bank).  Chunk 0 is cold (x still streaming
    from HBM): its first three tiles run kt-outer across 3 concurrent PSUM
    groups, consuming x quarters as they arrive.  vT transpose runs on the
    DMA XBAR (dma_start_transpose), not PE.
  - RoPE: rotate-half via partition-swap DMA; sin tables sign-folded, q
    tables pre-scaled by 1/sqrt(hd); k/q tables shared via stride-0
    broadcast APs; all products on DVE (2x bf16 path).
  - Scores transposed ST[j, i] = kT.T @ qT; exp on ACT; causal mask applied
    on PE as an additive -60 upper-tri bias matmul (m60 stationary x ident
    moving accumulated into the diagonal score block) — no vector-engine
    mask work and exp(masked) ~ e^-50 is harmless in the rowsums.
  - Softmax denominators: heads 0/3 (and all heads on short j-loops) use
    the classic all-ones stationary matmul accumulated alongside PV; on
    long j-loops heads 1/2 instead accumulate exp tiles elementwise (h1 on
    DVE; h2 split even/odd j-tiles across DVE and gpsimd) into f32 SBUF
    accumulators + a single 512-col ones-matmul partition reduce per head,
    software-pipelined into h3's j-loop.
  - Stage C accumulates 4 heads per (i-tile, ech) in PSUM (head order
    0,1,3,2 — h2's deferred normalize lands last), DVE-evicts 2-ech-packed
    bf16 tiles, DMAs on alternating queues; final tile goes per-ech so the
    drain only waits on a 128 KB transfer.  Host sums partials in f32.
  - DMA queues (both feed one HWDGE; a queue's sequencer shares its host
    engine, and a dep-waiting DMA at the head wedges that sequencer):
    sync/SP carries weights, rotate-half swaps, XBAR transposes, x for
    chunks 2/3 and half the outputs; scalar/ACT carries rope tables, x for
    chunks 0/1 (ACT idle then) and the other half of the outputs — so exp
    dispatch is never stuck behind a wedged descriptor ring.
  - PSUM budget (8 banks): psa 2 (A accumulators; B's deferred rowsum
    reduces) + st 3 (scores, shared with C's wo accumulators) + pv 2 + rs 1.
"""

import math
import os
from contextlib import ExitStack

import numpy as np

import concourse.bass as bass
import concourse.tile as tile
from concourse import bacc, mybir
from concourse.bass_utils import run_bass_kernel_spmd

# ---- problem shape (hardcoded per contract) ----
S = 2048           # seq len
D = 4096           # model dim
HD = 128           # head dim
N_CORES = 8
NQH = 4            # q heads per core
QCOLS = NQH * HD   # 512 wq columns per core
SC_N = 4           # seq chunks of 512
KT_N = D // 128    # 32 contraction tiles
JT_N = S // 128    # 16 key tiles
ECH_N = D // 512   # 8 output column chunks

F32 = mybir.dt.float32
BF16 = mybir.dt.bfloat16

MM_DT = BF16          # matmul operand dtype
USE_CHAINS = os.environ.get("K_CHAINS", "1") == "1"
PEP_BUFS = 12         # exp-tile ring (WAR slack for the DVE/Pool chains)

_BUILD_CACHE: dict = {}


def _make_pools(tc, ctx):
    def pool(name, bufs, space="SBUF"):
        return ctx.enter_context(tc.tile_pool(name=name, bufs=bufs, space=space))

    return {
        "wp": pool("w", 1),
        "xp": pool("x", 3),
        "rpp": pool("rope", 2),
        "kqp": pool("kq", 1),
        "rtp": pool("rt", 1),
        "pep": pool("pe", PEP_BUFS),
        "rcp": pool("rc", 2),
        "accp": pool("acc", 2),
        "obp": pool("ob", 3),
        "vfp": pool("vf", 2),
        "ps": pool("ps", 1, space="PSUM"),
    }


def _emit_body(nc, pools, dram, out, causal: bool):
    if True:
        wp = pools["wp"]
        xp = pools["xp"]
        rpp = pools["rpp"]
        kqp = pools["kqp"]
        rtp = pools["rtp"]
        pep = pools["pep"]
        rcp = pools["rcp"]
        accp = pools["accp"]
        obp = pools["obp"]
        vfp = pools["vfp"]
        ps = pools["ps"]

        # ---- resident weights & constants (DMA order = consumption order:
        # the cold-start A(0) phase consumes k/q0/q1 kt-outer immediately,
        # then q2, q3, v; rope 0/1 ride ahead of the big wo transfer) ----
        wk_sb = wp.tile([128, KT_N, HD], MM_DT, tag="wk", name="wk_sb")
        wq_sb = wp.tile([128, NQH, KT_N, HD], MM_DT, tag="wq", name="wq_sb")
        for i in range(4):
            ksl = slice(i * 8, (i + 1) * 8)
            nc.sync.dma_start(wk_sb[:, ksl, :], dram["wk"][:, ksl, :])
            nc.sync.dma_start(wq_sb[:, 0, ksl, :], dram["wq"][:, 0, ksl, :])
            nc.sync.dma_start(wq_sb[:, 1, ksl, :], dram["wq"][:, 1, ksl, :])
        nc.sync.dma_start(wq_sb[:, 2], dram["wq"][:, 2])
        nc.sync.dma_start(wq_sb[:, 3], dram["wq"][:, 3])
        wv_sb = wp.tile([128, KT_N, HD], MM_DT, tag="wv", name="wv_sb")
        nc.sync.dma_start(wv_sb[:], dram["wv"][:])
        ones_sb = wp.tile([128, 128], MM_DT, tag="ones", name="ones_sb")
        nc.sync.dma_start(ones_sb[:], dram["ones"][:])
        ident_sb = wp.tile([128, 128], MM_DT, tag="ident", name="ident_sb")
        nc.sync.dma_start(ident_sb[:], dram["ident"][:])
        # additive causal mask: m60[i, j] = -60 for j > i (else 0); applied on
        # PE as `st[:, blk] += m60.T @ ident` so no vector engine is involved
        m60_sb = wp.tile([128, 128], MM_DT, tag="m60", name="m60_sb")
        nc.sync.dma_start(m60_sb[:], dram["m60"][:])
        tri_sb = wp.tile([128, 128], MM_DT, tag="tri", name="tri_sb")
        nc.sync.dma_start(tri_sb[:], dram["tri"][:])

        # rope tables: rp[:, pl, t2, :] pl 0 = cos, 1 = sign-folded sin;
        # t2 0 = k table, 1 = q table (pre-scaled by 1/sqrt(hd)).
        # chunks 0/1 load ahead of wo; 2/3 load from their stage bodies
        rp_tiles = {}

        def load_rope(sc):
            rp = rpp.tile([HD, 2, 2, 512], MM_DT, tag="rp", name="rp")
            nc.scalar.dma_start(rp[:], dram["rope"][sc])
            rp_tiles[sc] = rp

        load_rope(0)
        load_rope(1)
        wo_sb = wp.tile([128, ECH_N, NQH, 512], MM_DT, tag="wo", name="wo_sb")
        for i in range(2):
            nc.sync.dma_start(
                wo_sb[:, i * 4:(i + 1) * 4, :, :], dram["wo"][:, i * 4:(i + 1) * 4, :, :]
            )

        # ---- per-chunk persistent activations ----
        # kqT[sc]: t=0 is kT, t=1..4 are qT (overwritten by attention output)
        kqT = [
            kqp.tile([128, 5, 512], MM_DT, tag=f"kq{sc}", name=f"kq{sc}")
            for sc in range(SC_N)
        ]
        v_sb = [
            kqp.tile([128, 4, HD], MM_DT, tag=f"v{sc}", name=f"v{sc}")
            for sc in range(SC_N)
        ]

        def stage_a(sc):
            # x for the chunk in two half-contraction tiles (bufs=3 gives a
            # half-chunk of prefetch ahead of compute)
            xlo = xp.tile([128, 16, 512], MM_DT, tag="xt", name="xlo")
            xhi = xp.tile([128, 16, 512], MM_DT, tag="xt", name="xhi")
            # chunks 0/1 stream x on the scalar queue (ACT is idle then);
            # later chunks ride the sync queue so their descriptor rings
            # never block exp dispatch on the ACT sequencer
            xq = nc.scalar if sc < 2 else nc.sync
            for i in range(4):
                xq.dma_start(
                    xlo[:, i * 4:(i + 1) * 4, :], dram["xn"][sc, i]
                )
            for i in range(4):
                xq.dma_start(
                    xhi[:, i * 4:(i + 1) * 4, :], dram["xn"][sc, 4 + i]
                )

            def xsl(kt):
                return (xlo if kt < 16 else xhi)[:, kt % 16, :]

            if sc not in rp_tiles:
                load_rope(sc)
            rp = rp_tiles[sc]

            def wsl(t, kt):
                return wk_sb[:, kt, :] if t == 0 else wq_sb[:, t - 1, kt, :]

            qc = rtp.tile([128, 5, 512], MM_DT, tag="qc", name="qc")
            # out-tile order: k, q0..q3 (rope group), then vT
            if sc == 0:
                # cold start: x is still streaming in, so pace the first
                # three tiles kt-outer (3 concurrent PSUM groups: psa x2 and
                # the otherwise-idle rs bank) to consume x as it arrives
                g = [
                    ps.tile([128, 512], F32, tag="psa", name="g0", bufs=2),
                    ps.tile([128, 512], F32, tag="psa", name="g1", bufs=2),
                    ps.tile([128, 512], F32, tag="rs", name="g2", bufs=1),
                ]
                # q0/q1 phase-shifted so the first matmuls only wait on wk's
                # first quarter, not on all three weight streams
                D1, D2 = 3, 6
                for kt in range(KT_N + D2):
                    if kt < KT_N:
                        nc.tensor.matmul(
                            g[0][:], wsl(0, kt), xsl(kt),
                            start=kt == 0, stop=kt == KT_N - 1,
                        )
                    if D1 <= kt < KT_N + D1:
                        kq = kt - D1
                        nc.tensor.matmul(
                            g[1][:], wsl(1, kq), xsl(kq),
                            start=kq == 0, stop=kq == KT_N - 1,
                        )
                    if D2 <= kt:
                        kq = kt - D2
                        nc.tensor.matmul(
                            g[2][:], wsl(2, kq), xsl(kq),
                            start=kq == 0, stop=kq == KT_N - 1,
                        )
                for t in range(3):
                    nc.vector.tensor_copy(qc[:, t, :], g[t][:])
                rest = range(3, 5)
            else:
                rest = range(5)
            for t in rest:
                pacc = ps.tile([128, 512], F32, tag="psa", name="psa", bufs=2)
                for kt in range(KT_N):
                    nc.tensor.matmul(
                        pacc[:],
                        wsl(t, kt),
                        xsl(kt),
                        start=kt == 0,
                        stop=kt == KT_N - 1,
                    )
                nc.vector.tensor_copy(qc[:, t, :], pacc[:])
            # rotate-half partition swap for all 5 tiles at once
            qs = rtp.tile([128, 5, 512], MM_DT, tag="qs", name="qs")
            nc.sync.dma_start(qs[0:64, :, :], qc[64:128, :, :])
            nc.sync.dma_start(qs[64:128, :, :], qc[0:64, :, :])
            # sin product (sign-folded tables) on DVE, cos product on gpsimd
            nc.vector.tensor_mul(qs[:, 0:1, :], qs[:, 0:1, :], rp[:, 1, 0:1, :])
            nc.vector.tensor_mul(
                qs[:, 1:5, :], qs[:, 1:5, :],
                rp[:, 1, 1:2, :].broadcast_to([HD, 4, 512]),
            )
            nc.vector.tensor_mul(qc[:, 0:1, :], qc[:, 0:1, :], rp[:, 0, 0:1, :])
            nc.vector.tensor_mul(
                qc[:, 1:5, :], qc[:, 1:5, :],
                rp[:, 0, 1:2, :].broadcast_to([HD, 4, 512]),
            )
            nc.vector.tensor_add(kqT[sc][:], qc[:], qs[:])

            # vT projection + transpose to v_sb[sc] ([j, d] layout) — the
            # transpose runs on the DMA XBAR, not the tensor engine
            pv = ps.tile([128, 512], F32, tag="psa", name="psav", bufs=2)
            for kt in range(KT_N):
                nc.tensor.matmul(
                    pv[:], wv_sb[:, kt, :], xsl(kt),
                    start=kt == 0, stop=kt == KT_N - 1,
                )
            vt_f = vfp.tile([128, 512], MM_DT, tag="vt", name="vt")
            nc.vector.tensor_copy(vt_f[:], pv[:])
            for vi in range(4):
                nc.sync.dma_start_transpose(
                    v_sb[sc][:, vi, :], vt_f[:, vi * 128:(vi + 1) * 128]
                )

        def stage_b(ic, cth=None):
            njt = 4 * (ic + 1) if causal else JT_N
            # chains only pay off on long j-loops; short chunks use the
            # classic per-j-tile PE rowsum for every head
            chains = USE_CHAINS and njt >= 12
            # interleave the previous chunk's wo-matmul groups into the
            # chain heads' j-loops: those are exp(ACT)-bound, so the pc
            # matmuls fill otherwise-idle PE cycles
            cth = list(cth) if cth else []
            cstride = 1
            cslot = [0]

            def pop_c():
                if cth:
                    cth.pop(0)()
            q12 = max(njt // 2, 2)
            q34 = max(3 * njt // 4, 3)
            # h1/h2 exp accumulators (f32) + staging
            acc1 = accp.tile([128, 512], F32, tag="acc", name="acc1")
            acc2e = accp.tile([128, 512], F32, tag="acc", name="acc2e")
            acc2o = accp.tile([128, 512], F32, tag="acco", name="acc2o", bufs=1)
            accb1 = accp.tile([128, 512], MM_DT, tag="accb", name="accb1")
            accb2 = accp.tile([128, 512], MM_DT, tag="accb", name="accb2")
            pvf1 = accp.tile([128, 512], F32, tag="pvf", name="pvf1")
            pvf2 = accp.tile([128, 512], F32, tag="pvf", name="pvf2")
            rc = [None] * NQH
            rsp = [None] * NQH
            # off of jt==1 (odd-subchain init) — nonzero only for ic==0
            co = 128 if (causal and ic == 0) else 0

            def mk_rc(h):
                t = rcp.tile([128, 512], F32, tag="rc", name=f"rc{h}")
                rc[h] = t
                return t

            def fin_reduce(h):
                # ones-matmul partition reduce of the accumulated exp sums
                accb = accb1 if h == 1 else accb2
                r = ps.tile([128, 512], F32, tag="psa", name=f"rsp{h}", bufs=2)
                rsp[h] = r
                nc.tensor.matmul(r[:], ones_sb[:], accb[:], start=True, stop=True)

            def fin_norm(h):
                nc.vector.reciprocal(mk_rc(h)[:], rsp[h][:])
                pvf = pvf1 if h == 1 else pvf2
                nc.vector.tensor_mul(kqT[ic][:, 1 + h, :], pvf[:], rc[h][:])

            def emit_point(h, jt):
                if not chains:
                    return
                if h == 2 and jt == 1:
                    # h1 chain fully emitted -> DVE-in-order guarantees done;
                    # early emission keeps it ahead of h2's chain backlog
                    nc.vector.tensor_copy(accb1[:], acc1[:])
                elif h == 3:
                    if jt == 0:
                        fin_reduce(1)
                    elif jt == 1:
                        fin_norm(1)
                    elif jt == q12:
                        # combine h2 odd subchain into even, convert
                        nc.vector.tensor_add(
                            acc2e[:, co:], acc2e[:, co:], acc2o[:, co:]
                        )
                        nc.vector.tensor_copy(accb2[:], acc2e[:])
                    elif jt == q34:
                        fin_reduce(2)
                        if q34 + 1 > njt - 1:
                            fin_norm(2)
                    elif jt == q34 + 1:
                        fin_norm(2)

            for h in range(NQH):
                classic = not chains or h in (0, 3)
                pv = ps.tile([128, 512], F32, tag="pv", name=f"pv{h}", bufs=2)
                rs = (
                    ps.tile([128, 512], F32, tag="rs", name=f"rs{h}", bufs=1)
                    if classic
                    else None
                )
                for jt in range(njt):
                    dt_ = jt - 4 * ic  # diagonal tile index (causal only)
                    diag = causal and dt_ >= 0
                    off = 128 * dt_ if (causal and dt_ > 0) else 0
                    blk = off + 128
                    kT = kqT[jt // 4][:, 0, (jt % 4) * 128:(jt % 4 + 1) * 128]
                    qT = kqT[ic][:, 1 + h, :]
                    st = ps.tile([128, 512], F32, tag="st", name="st", bufs=3)
                    if diag and USE_CHAINS:
                        # causal mask as additive -60 upper-tri bias on PE:
                        # score over the full range, mask accumulated onto the
                        # diagonal block only
                        nc.tensor.matmul(
                            st[:, off:], kT, qT[:, off:],
                            start=True, stop=False,
                        )
                        nc.tensor.matmul(
                            st[:, off:blk], m60_sb[:], ident_sb[:],
                            start=False, stop=True, skip_group_check=True,
                        )
                    else:
                        nc.tensor.matmul(
                            st[:, off:], kT, qT[:, off:], start=True, stop=True
                        )
                    pe = pep.tile([128, 512], MM_DT, tag="pe", name="pe")
                    nc.scalar.activation(
                        pe[:, off:], st[:, off:], mybir.ActivationFunctionType.Exp
                    )
                    if not classic:
                        # absorbed wo-matmul work sits between the score and
                        # the PV matmul, hiding the exp latency
                        cslot[0] += 1
                        if cslot[0] % cstride == 0:
                            pop_c()
                    if diag and not USE_CHAINS:
                        nc.vector.tensor_mul(
                            pe[:, off:blk], pe[:, off:blk], tri_sb[:]
                        )
                    first, last = jt == 0, jt == njt - 1
                    nc.tensor.matmul(
                        pv[:, off:], v_sb[jt // 4][:, jt % 4, :], pe[:, off:],
                        start=first, stop=last,
                    )
                    if classic:
                        nc.tensor.matmul(
                            rs[:, off:], ones_sb[:], pe[:, off:],
                            start=first, stop=last,
                        )
                    elif h == 1:
                        if first:
                            nc.vector.tensor_copy(acc1[:], pe[:])
                        else:
                            nc.vector.tensor_add(
                                acc1[:, off:], acc1[:, off:], pe[:, off:]
                            )
                    else:  # h == 2: even jts on DVE, odd jts on gpsimd
                        tgt, eng = (
                            (acc2e, nc.vector) if jt % 2 == 0 else (acc2o, nc.gpsimd)
                        )
                        if jt < 2:
                            eng.tensor_copy(tgt[:, off:], pe[:, off:])
                        else:
                            eng.tensor_add(
                                tgt[:, off:], tgt[:, off:], pe[:, off:]
                            )
                    emit_point(h, jt)
                # end of h's j-loop
                if classic:
                    nc.vector.reciprocal(mk_rc(h)[:], rs[:])
                    nc.vector.tensor_mul(kqT[ic][:, 1 + h, :], pv[:], rc[h][:])
                elif h == 1:
                    nc.vector.tensor_copy(pvf1[:], pv[:])
                else:
                    nc.vector.tensor_copy(pvf2[:], pv[:])
            # flush any wo-matmul groups not absorbed by the j-loops
            while cth:
                cth.pop(0)()

        def c_thunks(ic):
            # per-pc granularity: one thunk = one 4-head wo accumulation +
            # eviction, so each exp-bound B slot can absorb exactly one
            thunks = []
            for itl in range(4):
                it = 4 * ic + itl
                isl = slice(itl * 128, (itl + 1) * 128)
                # the final tile's output DMAs go out per-ech so the end-of-
                # kernel drain only waits on a 128 KB transfer
                fine = causal and ic == SC_N - 1 and itl == 3
                for eh in range(ECH_N // 2):
                    # pack 2 ech per ob tile; DVE eviction (gpsimd can't read
                    # PSUM), one DMA per pair on alternating queues
                    holder = {}

                    def th(e2, it=it, isl=isl, eh=eh, fine=fine, holder=holder):
                        if e2 == 0:
                            holder["ob"] = obp.tile(
                                [128, 2, 512], MM_DT, tag="ob", name="ob"
                            )
                        ob = holder["ob"]
                        ech = 2 * eh + e2
                        pc = ps.tile(
                            [128, 512], F32, tag="st", name="pc", bufs=3
                        )
                        # h2's softmax finalize lands latest -> read last
                        for hx, h in enumerate((0, 1, 3, 2)):
                            nc.tensor.matmul(
                                pc[:],
                                kqT[ic][:, 1 + h, isl],
                                wo_sb[:, ech, h, :],
                                start=hx == 0,
                                stop=hx == NQH - 1,
                            )
                        nc.vector.tensor_copy(ob[:, e2, :], pc[:])
                        if fine:
                            deng = nc.sync if (it + ech) % 2 else nc.scalar
                            deng.dma_start(out[it, eh][:, e2, :], ob[:, e2, :])
                        elif e2 == 1:
                            deng = nc.sync if (it + eh) % 2 else nc.scalar
                            deng.dma_start(out[it, eh], ob[:])

                    from functools import partial
                    thunks.append(partial(th, 0))
                    thunks.append(partial(th, 1))
            return thunks

        def stage_c(ic):
            for th in c_thunks(ic):
                th()

        if causal:
            # C1/C2 are absorbed into the exp-bound stretches of B2/B3
            stage_a(0)
            stage_a(1)
            stage_b(0)
            stage_c(0)
            stage_a(2)
            stage_b(1)
            stage_a(3)
            stage_b(2, c_thunks(1))
            stage_b(3, c_thunks(2))
            stage_c(3)
        else:
            # non-causal: every B chunk reads all chunks' K/V, so all of
            # stage A must be emitted first (a B read of a not-yet-written
            # kqT range would get no RAW dependency from Tile)
            for sc in range(SC_N):
                stage_a(sc)
            for ic in range(SC_N):
                stage_b(ic)
                stage_c(ic)


def build_nc(causal: bool = True, reps: int = 1):
    nc = bacc.Bacc(
        "TRN2", target_bir_lowering=False, debug=False, num_devices=N_CORES
    )
    dram = {}
    for name, shape, dt in [
        # host-prepermuted layouts: every DMA reads/writes contiguous
        # per-partition runs
        ("xn", [SC_N, 8, 128, 4, 512], MM_DT),
        ("wq", [128, NQH, KT_N, HD], MM_DT),
        ("wk", [128, KT_N, HD], MM_DT),
        ("wv", [128, KT_N, HD], MM_DT),
        ("wo", [128, ECH_N, NQH, 512], MM_DT),
        ("rope", [SC_N, HD, 2, 2, 512], MM_DT),
        ("ones", [128, 128], MM_DT),
        ("ident", [128, 128], MM_DT),
        ("m60", [128, 128], MM_DT),
        ("tri", [128, 128], MM_DT),
    ]:
        dram[name] = nc.dram_tensor(name, shape, dt, kind="ExternalInput").ap()
    out = nc.dram_tensor("out", [JT_N, ECH_N // 2, 128, 2, 512], MM_DT,
                         kind="ExternalOutput").ap()

    with tile.TileContext(nc) as tc:
        with ExitStack() as ctx:
            pools = _make_pools(tc, ctx)
            for _ in range(reps):
                _emit_body(nc, pools, dram, out, causal)

    nc.compile()
    return nc


def get_nc(causal: bool = True):
    if causal not in _BUILD_CACHE:
        _BUILD_CACHE[causal] = build_nc(causal)
    return _BUILD_CACHE[causal]


def _mm_np(a):
    return np.ascontiguousarray(a).astype(mybir.dt.np(MM_DT))


def prep_in_maps(x, sincos, wq, wk, wv, wo):
    """Host-side shard + layout prep. Returns list of per-core input dicts.

    All tensors are pre-permuted so that every device DMA moves contiguous
    per-partition runs (device DMA engines are far more efficient that way).
    """
    x = np.asarray(x, np.float32)
    assert x.shape == (1, S, D)
    # xn[sc, xg, p, k4, n] = x[sc*512 + n, (xg*4 + k4)*128 + p]
    xn = _mm_np(
        x[0].reshape(SC_N, 512, 8, 4, 128).transpose(0, 2, 4, 3, 1)
    )

    sincos = np.asarray(sincos, np.float32)
    sin = sincos[:S, :HD]
    cos = sincos[:S, HD:]
    sinT = np.ascontiguousarray(sin.T)
    cosT = np.ascontiguousarray(cos.T)
    sin_sgn = sinT.copy()
    sin_sgn[:64] = -sinT[:64]
    scale = np.float32(1.0 / math.sqrt(HD))
    # rope[sc, d, pl, t2, n]: pl 0 = cos, 1 = sin(signed); t2 0 = k, 1 = q
    base = np.stack([cosT, sin_sgn], axis=0)           # [2(pl), HD, S]
    rope = np.stack([base, base * scale], axis=1)      # [2(pl), 2(t2), HD, S]
    rope = rope.reshape(2, 2, HD, SC_N, 512).transpose(3, 2, 0, 1, 4)
    rope = _mm_np(rope)

    wq = np.asarray(wq, np.float32)
    wk = np.asarray(wk, np.float32)
    wv = np.asarray(wv, np.float32)
    wo = np.asarray(wo, np.float32)

    ones = _mm_np(np.ones((128, 128), np.float32))
    ident = _mm_np(np.eye(128, dtype=np.float32))
    m60 = _mm_np(np.triu(np.ones((128, 128), np.float32), 1) * np.float32(-60.0))
    tri = _mm_np(np.triu(np.ones((128, 128), np.float32)))

    in_maps = []
    for c in range(N_CORES):
        wq_c = wq[:, c * QCOLS:(c + 1) * QCOLS]          # [D, 512]
        wk_c = wk[:, c * HD:(c + 1) * HD]                # [D, 128]
        wv_c = wv[:, c * HD:(c + 1) * HD]
        wo_c = wo[c * QCOLS:(c + 1) * QCOLS, :]          # [512, D]
        in_maps.append(
            {
                "xn": xn,
                # wq[p, h, kt, d] = wq_c[kt*128 + p, h*HD + d]
                "wq": _mm_np(
                    wq_c.reshape(KT_N, 128, NQH, HD).transpose(1, 2, 0, 3)
                ),
                "wk": _mm_np(
                    wk_c.reshape(KT_N, 128, HD).transpose(1, 0, 2)
                ),
                "wv": _mm_np(
                    wv_c.reshape(KT_N, 128, HD).transpose(1, 0, 2)
                ),
                # wo[p, ech, h, n] = wo_c[h*128 + p, ech*512 + n]
                "wo": _mm_np(
                    wo_c.reshape(NQH, 128, ECH_N, 512).transpose(1, 2, 0, 3)
                ),
                "rope": rope,
                "ones": ones,
                "ident": ident,
                "m60": m60,
                "tri": tri,
            }
        )
    return in_maps


def unpermute_out(out_n):
    """out_n [it, eh, p, e2, n] -> out [S, D]."""
    return np.ascontiguousarray(
        out_n.transpose(0, 2, 1, 3, 4).reshape(S, D)
    )


def check_mask(full_causal_mask, start_pos) -> bool:
    """Returns True for causal (tril) mask, False for all-allowed."""
    sp = int(start_pos)
    assert sp == 0, f"kernel specialized for start_pos=0, got {sp}"
    m = np.asarray(full_causal_mask)
    assert m.shape == (1, 1, S, S)
    m = m[0, 0]
    tril = np.tril(np.ones((S, S), dtype=bool))
    if (m == tril).all():
        return True
    if m.all():
        return False
    raise AssertionError("unsupported mask pattern")


def kernel(
    x,
    start_pos,
    sincos,
    full_causal_mask,
    wq,
    wk,
    wv,
    wo,
    cache_k,
    cache_v,
):
    causal = check_mask(full_causal_mask, start_pos)
    # cache_k/cache_v are zero and fully overwritten in the attended region
    # (start_pos=0, seq_len == max_seq_len) — they do not affect the output.
    nc = get_nc(causal)
    in_maps = prep_in_maps(x, sincos, wq, wk, wv, wo)
    res = run_bass_kernel_spmd(nc, in_maps, list(range(N_CORES)))
    acc = res.results[0]["out"].astype(np.float32)
    for c in range(1, N_CORES):
        acc = acc + res.results[c]["out"].astype(np.float32)
    return unpermute_out(acc)[np.newaxis]
